# revision 53
# baseline (speedup 1.0000x reference)
"""Trainium2 Bass kernel for the binarized CNN:
conv3x3(sign weights) -> BN -> ternary hardtanh -> maxpool4 -> linear(sign weights)

Strategy (pure data parallel over batch, 8 cores x 512 samples):
  - Host builds the im2col matrix (numpy as_strided view of x.T) and
    splits it into an exact fp16 hi/lo pair, so the conv runs as TWO
    accumulating fp16 matmuls per tile against EXACT +-1 sign weights:
    1 cyc/col on the PE (4x the fp32 rate) with fp32-level accuracy
    (hi+lo reconstructs x to ~2^-23; +-1 weights are exact in fp16).
  - The BN affine is NOT folded into the conv (that would break fp16
    exactness); instead maxpool commutes with the monotone affine, so
    pooling runs on RAW sign-conv outputs, Pool (gpsimd) applies the
    affine to the 16x-smaller pooled tile, and ternary is two Act-engine
    Signs: t2 = sign(y - .5) + sign(y + .5) = 2t (FC weights halved).
  - HW legality on TRN2: only DVE can max, with at most one PSUM
    operand per op. The Act engine copies z(h0) to SBUF; DVE chains
    tt-maxes that each ingest one PSUM z alongside the running SBUF max
    (2 operand reads per cycle), then strided tt w-pooling. Pool
    (gpsimd) handles the mask subtract; emission is software-pipelined
    one group deep so the in-order queues never block cross-engine.
  - FC inputs become feature-major via ONE dma_start_transpose per
    (batch-tile, h-group) with a 3D dest AP; 27 small accumulating bf16
    matmuls (plus a K=1 fp32 bias matmul) run interleaved per group.
    Output stays [10, 512] on device; host transposes.
"""

import numpy as np
from contextlib import ExitStack

import concourse.bass as bass
import concourse.tile as tile
from concourse import bacc, mybir
from concourse.bass_utils import run_bass_kernel_spmd

F32 = mybir.dt.float32
F16 = mybir.dt.float16
BF16 = mybir.dt.bfloat16
ALU = mybir.AluOpType

NCORES = 8
# (the baseline's LDWEIGHTS walrus patch is gone: fp16 matmuls use the
# background weight buffer, so redundant-load dedup no longer matters)
BFULL = 4096
B = BFULL // NCORES          # 512 per core
P = 128
BT = B // P                  # 4 batch tiles
H, W = 14, 38
HO, WO = 12, 36
C = 32
HW = H * W                   # 532
KP = 3 * W                   # 114 patch rows (no bias row: thresholds carry it)
NF = C * WO                  # 1152 conv outputs per (b, h)
CW3 = C * (WO // 4)          # 288 after w-pool
EPS = 1e-5
NOUT = 10
NJJ = 9                      # FC feature blocks (3 h3 * 3 blocks of 128)


def _host_prep(conv_w, conv_b, bn_gamma, bn_beta, bn_mean, bn_var, fc_w, fc_b):
    import ml_dtypes
    inv = (bn_gamma / np.sqrt(bn_var + EPS)).astype(np.float32)
    tb = ((conv_b - bn_mean) * inv + bn_beta).astype(np.float32)
    sw = np.sign(conv_w[:, 0]).astype(np.float32)          # [32, 3, 3]

    # Toeplitz sign-weight matrix, exact in fp16
    wt = np.zeros((KP, NF), np.float32)
    for c in range(C):
        for w in range(WO):
            n = c * WO + w
            for i in range(3):
                for j in range(3):
                    wt[i * W + w + j, n] = sw[c, i, j]

    # BN affine applied to the POOLED u (pool commutes: inv > 0):
    # y = u*inv[c] + tb[c], per conv column in w-pooled layout
    aff = np.zeros((P, 2 * CW3), np.float32)
    for c in range(C):
        for w3 in range(WO // 4):
            n = c * 9 + w3
            aff[:, n] = inv[c]
            aff[:, CW3 + n] = tb[c]

    # FC carries t2 = sign(y - .5) + sign(y + .5) = 2t exactly, so weights
    # are halved (exact in bf16) and the bias needs no correction.
    sf = np.sign(fc_w).astype(np.float32)                  # [10, 864]
    sfc = np.zeros((P, NJJ * NOUT), np.float32)
    for jj in range(NJJ):
        h3, j = jj // 3, jj % 3
        for p in range(P):
            col = j * P + p                                # t-tile column
            if col < CW3:
                c, w3 = col // 9, col % 9
                f = c * 27 + h3 * 9 + w3                   # reference flatten order
                sfc[p, jj * NOUT:(jj + 1) * NOUT] = 0.5 * sf[:, f]

    fcb = fc_b.astype(np.float32).reshape(1, NOUT)
    return (wt.astype(np.float16), aff,
            sfc.astype(ml_dtypes.bfloat16), fcb)


def _host_im2col(x):
    """x [BFULL, 532] -> exact fp16 hi/lo transposed pair [532, BFULL];
    the device im2col DMAs read overlapping 114-row windows of these."""
    xT = np.ascontiguousarray(x.T)                         # [532, BFULL] f32
    xh = xT.astype(np.float16)
    xl = (xT - xh.astype(np.float32)).astype(np.float16)
    return xh, xl


def _build():
    nc = bacc.Bacc("TRN2", target_bir_lowering=False, debug=False,
                   num_devices=NCORES)
    xh_d = nc.dram_tensor("xh", [HW, B], F16, kind="ExternalInput").ap()
    xl_d = nc.dram_tensor("xl", [HW, B], F16, kind="ExternalInput").ap()
    wt_d = nc.dram_tensor("wt", [KP, NF], F16, kind="ExternalInput").ap()
    aff_d = nc.dram_tensor("aff", [P, 2 * CW3], F32, kind="ExternalInput").ap()
    sfc_d = nc.dram_tensor("sfc", [P, NJJ * NOUT], BF16, kind="ExternalInput").ap()
    fcb_d = nc.dram_tensor("fcb", [1, NOUT], F32, kind="ExternalInput").ap()
    out_d = nc.dram_tensor("out", [NOUT, B], F32, kind="ExternalOutput").ap()

    with tile.TileContext(nc) as tc, ExitStack() as ctx:
        const = ctx.enter_context(tc.tile_pool(name="const", bufs=1))
        abp = ctx.enter_context(tc.tile_pool(name="ab", bufs=3))
        mp = ctx.enter_context(tc.tile_pool(name="m", bufs=3))
        up = ctx.enter_context(tc.tile_pool(name="u", bufs=3))
        lp = ctx.enter_context(tc.tile_pool(name="l", bufs=3))

        wt = const.tile([KP, NF], F16, tag="wt")
        aff = const.tile([P, 2 * CW3], F32, tag="aff")
        sfc = const.tile([P, NJJ * NOUT], BF16, tag="sfc")
        fcb = const.tile([1, NOUT], F32, tag="fcb")
        imh = const.tile([KP, HO * B], F16, tag="imh")
        iml = const.tile([KP, HO * B], F16, tag="iml")

        # im2col loads in 3 waves of ONE strided DMA each per tensor: the
        # DRAM side reads overlapping 114-row windows of xT (rows 38h..
        # 38h+114), so the host ships the compact transpose, not the
        # 2.6x-inflated im2col. Group 0's columns (h0-h3, bt0) go first,
        # then their remaining batch tiles, then the h4-h11 bulk.
        from bass_rust import VecI64Pair

        def imv(t, h0, h1, b0, b1):
            return t[:].rearrange("p (h b) -> p h b", h=HO)[:, h0:h1, b0:b1]

        def im_src(s_d, h0, h1, b0, b1):
            # overlapping-window AP: (r, h, b) -> xT[38*(h0+h) + r, b0+b]
            v = s_d[38 * h0:38 * h0 + KP, b0:b1].copy()
            v.ap = VecI64Pair([[B, KP], [38 * B, h1 - h0], [1, b1 - b0]])
            return v

        # all on the sync queue: the Act queue must stay free for the
        # per-group PSUM->SBUF copies from t~3us on
        for wave, (h0, h1, b0, b1) in enumerate(
                ((0, 4, 0, P), (0, 4, P, B), (4, HO, 0, B))):
            for dst, s_d in ((imh, xh_d), (iml, xl_d)):
                nc.sync.dma_start(imv(dst, h0, h1, b0, b1),
                                  im_src(s_d, h0, h1, b0, b1))
            if wave == 0:
                nc.sync.dma_start(wt[:], wt_d)
            elif wave == 1:
                nc.sync.dma_start(aff[:], aff_d)
                nc.sync.dma_start(sfc[:], sfc_d)
                nc.sync.dma_start(fcb[:], fcb_d)

        onesr = const.tile([1, B], F32, tag="onesr")
        nc.gpsimd.memset(onesr[:], 1.0)
        halfb = const.tile([P, 1], F32, tag="halfb")
        nc.gpsimd.memset(halfb[:], 0.5)
        neghb = const.tile([P, 1], F32, tag="neghb")
        nc.gpsimd.memset(neghb[:], -0.5)
        # preload the act table (covers Copy and Sign) before first use
        sscr = const.tile([1, 1], F32, tag="sscr")
        nc.scalar.copy(sscr[:], onesr[0:1, 0:1])

        # ternary tiles: 3 rotating buffers, pad cols [288:384) zeroed once
        tts = [const.tile([P, 3 * P], BF16, tag=f"tt{i}", name=f"tt{i}")
               for i in range(3)]
        for t_ in tts:
            nc.gpsimd.memset(t_[:, CW3:3 * P], 0.0)

        # transposed FC input [128, (jj, b)]
        tT = const.tile([P, NJJ * B], BF16, tag="tT")

        with tc.tile_pool(name="zp", bufs=1, space="PSUM") as zp, \
             tc.tile_pool(name="fcp", bufs=1, space="PSUM") as fcp:
            acc = fcp.tile([NOUT, B], F32, tag="acc")
            groups = [(h3, bt) for h3 in range(3) for bt in range(BT)]
            tails = {}

            def emit_front(gidx):
                # HW legality: only DVE can max, with at most ONE PSUM
                # operand per op. Act copies z(h0) to SBUF, then DVE chains
                # tt-maxes that each ingest one PSUM z alongside the running
                # SBUF max (2 reads/cycle).
                h3, bt = groups[gidx]
                zs = {}
                for hh in range(4):
                    h = h3 * 4 + hh
                    k = h * 4 + bt
                    z = zp.tile([P, 1536], F32, tag=f"z{hh % 2}",
                                name=f"z{hh % 2}")
                    for part, im in ((0, imh), (1, iml)):
                        lhs = im[:, k * P:(k + 1) * P]
                        for c in range(3):
                            nc.tensor.matmul(
                                z[:, c * 512:c * 512 + 384], lhsT=lhs,
                                rhs=wt[:, c * 384:(c + 1) * 384],
                                start=(part == 0), stop=(part == 1))
                    zs[hh] = z

                def zv(t):   # compact [p, 3, 384] view of a 3-bank z tile
                    return t[:].rearrange("p (c q) -> p c q", q=512)[:, :, 0:384]

                def cvw(t):  # compact [p, 3, 384] view of an SBUF tile
                    return t[:].rearrange("p (c q) -> p c q", q=384)

                z0s = abp.tile([P, NF], F32, tag="z0s", name="z0s")
                nc.scalar.copy(cvw(z0s), zv(zs[0]))
                tails[gidx] = (zs, z0s, zv, cvw)

            def emit_chain(gidx):
                zs, z0s, zv, cvw = tails[gidx]
                c1 = mp.tile([P, NF], F32, tag="c1", name="c1")
                nc.vector.tensor_max(cvw(c1), zv(zs[1]), cvw(z0s))
                c2 = mp.tile([P, NF], F32, tag="c2", name="c2")
                nc.vector.tensor_max(cvw(c2), zv(zs[2]), cvw(c1))
                c3 = abp.tile([P, NF], F32, tag="c3", name="c3")
                nc.vector.tensor_max(cvw(c3), zv(zs[3]), cvw(c2))
                tails[gidx] = c3

            def emit_back(gidx):
                h3, bt = groups[gidx]
                c3 = tails.pop(gidx)
                m1 = mp.tile([P, 2 * CW3], F32, tag="m1", name="m1")
                cv = c3[:].rearrange("p (g w) -> p g w", w=4)
                nc.vector.tensor_max(
                    m1[:].rearrange("p (g w) -> p g w", w=2),
                    cv[:, :, 0:2], cv[:, :, 2:4])
                u = up.tile([P, CW3], F32, tag="u", name="u")
                mv = m1[:].rearrange("p (g w) -> p g w", w=2)
                nc.vector.tensor_max(u[:], mv[:, :, 0], mv[:, :, 1])
                # BN affine on the pooled u runs on Pool (mul/add are its
                # only legal tensor-tensor ops); ternary as two Act Signs:
                # t2 = sign(y - .5) + sign(y + .5) = 2t
                ya = lp.tile([P, CW3], F32, tag="ya", name="ya")
                nc.gpsimd.tensor_mul(ya[:], u[:], aff[:, 0:CW3])
                yb = lp.tile([P, CW3], F32, tag="yb", name="yb")
                nc.gpsimd.tensor_add(yb[:], ya[:], aff[:, CW3:2 * CW3])
                t_ = tts[gidx % 3]
                if gidx < len(groups) - 1:
                    s1 = lp.tile([P, CW3], F32, tag="s1", name="s1")
                    nc.scalar.activation(s1[:], yb[:],
                                         mybir.ActivationFunctionType.Sign,
                                         bias=neghb[:, 0:1], scale=1.0)
                    s2 = lp.tile([P, CW3], F32, tag="s2", name="s2")
                    nc.scalar.activation(s2[:], yb[:],
                                         mybir.ActivationFunctionType.Sign,
                                         bias=halfb[:, 0:1], scale=1.0)
                    nc.gpsimd.tensor_add(t_[:, 0:CW3], s1[:], s2[:])
                else:
                    # final group sits on the exit critical path: scalar-
                    # threshold compares on DVE beat two serial Act passes
                    s1 = lp.tile([P, CW3], F32, tag="s1", name="s1")
                    nc.vector.tensor_scalar(s1[:], yb[:], 0.5, 2.0,
                                            ALU.is_gt, ALU.mult)
                    s2 = lp.tile([P, CW3], F32, tag="s2", name="s2")
                    nc.vector.tensor_scalar(s2[:], yb[:], -0.5, 2.0,
                                            ALU.is_lt, ALU.mult)
                    nc.vector.tensor_sub(t_[:, 0:CW3], s1[:], s2[:])
                eng = nc.scalar
                eng.dma_start_transpose(
                    tT[:].rearrange("p (jj b) -> p jj b", jj=NJJ)
                        [:, h3 * 3:(h3 + 1) * 3, bt * P:(bt + 1) * P],
                    t_[:])

            ob = const.tile([NOUT, B], F32, tag="ob")

            def fc_emit(gidx):
                # 3 accumulating [10, 128] matmuls for this group's batch
                # tile; the group's transpose finished ~1 group ago
                h3, bt = groups[gidx]
                for j in range(3):
                    jj = h3 * 3 + j
                    nc.tensor.matmul(
                        acc[:, bt * P:(bt + 1) * P],
                        lhsT=sfc[:, jj * NOUT:(jj + 1) * NOUT],
                        rhs=tT[:].rearrange("p (jj b) -> p jj b", jj=NJJ)
                            [:, jj, bt * P:(bt + 1) * P],
                        start=False,
                        stop=(gidx == len(groups) - 1 and j == 2))

            def out_emit():
                # acc is readable only once the accumulation group stops;
                # split the PSUM->SBUF copy and the store across engines
                nc.scalar.copy(ob[:, 0:256], acc[:, 0:256])
                nc.vector.tensor_copy(ob[:, 256:B], acc[:, 256:B])
                nc.sync.dma_start(out_d[:, 0:256], ob[:, 0:256])
                nc.scalar.dma_start(out_d[:, 256:B], ob[:, 256:B])

            ngroups = len(groups)
            for g in range(ngroups):
                emit_front(g)
                if g >= 1:
                    emit_back(g - 1)
                emit_chain(g)
                if g >= 2:
                    if g == 2:
                        # bias via a K=1 fp32 matmul opens the accumulation
                        # group, so the tail needs no bias pass over acc
                        nc.tensor.matmul(acc[:, :], lhsT=fcb[:], rhs=onesr[:],
                                         start=True, stop=False)
                    fc_emit(g - 2)
            emit_back(ngroups - 1)
            fc_emit(ngroups - 2)
            fc_emit(ngroups - 1)
            out_emit()

    nc.compile()
    return nc


_NC_CACHE = None


def kernel(x, conv_w, conv_b, bn_gamma, bn_beta, bn_mean, bn_var, fc_w, fc_b):
    global _NC_CACHE
    x = np.asarray(x, np.float32).reshape(BFULL, HW)
    xh, xl = _host_im2col(x)                               # [532, BFULL] f16
    wt, aff, sfc, fcb = _host_prep(
        np.asarray(conv_w, np.float32), np.asarray(conv_b, np.float32),
        np.asarray(bn_gamma, np.float32), np.asarray(bn_beta, np.float32),
        np.asarray(bn_mean, np.float32), np.asarray(bn_var, np.float32),
        np.asarray(fc_w, np.float32), np.asarray(fc_b, np.float32))

    if _NC_CACHE is None:
        _NC_CACHE = _build()
    nc = _NC_CACHE

    in_maps = [
        dict(xh=np.ascontiguousarray(xh[:, i * B:(i + 1) * B]),
             xl=np.ascontiguousarray(xl[:, i * B:(i + 1) * B]),
             wt=wt, aff=aff, sfc=sfc, fcb=fcb)
        for i in range(NCORES)
    ]
    res = run_bass_kernel_spmd(nc, in_maps, core_ids=list(range(NCORES)))
    out = np.concatenate(
        [res.results[i]["out"].T for i in range(NCORES)], axis=0)
    return out.astype(np.float32)
